# revision 47
# baseline (speedup 1.0000x reference)
"""Multi-head attention kernel for Trainium2 (8 NeuronCores, SPMD).

Problem: x [4,1,2048,3], W_query/W_key/W_value [1,8,3,3] ->
ctx [4,8,2048,3] = softmax((x Wq)(x Wk)^T / sqrt(3)) @ (x Wv), returned
as a (ctx, ctx) tuple matching the reference.

Sharding: 32 (batch, head) blocks over 8 cores -> core c owns batch c//2,
heads 4*(c%2) .. +4. Each core runs an identical Bass program on its slice.

Per-core device program (S=2048; 2 head-pairs x 4 query chunks of 512;
per chunk 16 buffers of [128 keys, 2x512] = one key tile, both heads):

  - exp is the element roofline (4*2048*2048 elems), SPLIT between ACT
    (true exp, odd buffers, 1 elem/lane/cycle @1.2GHz) and DVE (even
    buffers; Schraudolph bit-trick: one tensor_scalar computes
    round(s*C1+C2) into an int16 tile whose bf16 bitcast IS ~exp(s),
    <=3.3% per-element, HW-verified round-to-nearest; end-to-end rel
    err ~7.6e-3 vs the 2e-2 gate). Alternating buffers keeps both
    engines running the softmax concurrently.

  - PSUM: s pool = 3 x [128, 1024] f32 (banks 0-5, TRIPLE buffered so
    QK(b+3) <- exp(b) slot reuse never binds the pipeline);
    ctx = one persistent [128, 1024] tile (banks 6-7) per chunk.

  - PE: ~6 full-row dummy matmuls on a memset tile run during the DMA
    lead-in to flip the HAM clock gate to 8/8 (a cold start measured
    ~26us at 1.2GHz). QK per unit is [32,128]x[32,512] row-tiled; each
    head's 18 stacked rows are DUPLICATED into two 32-row groups
    (64*grp(t%2) + 32*hl) so consecutive buffers hit disjoint groups
    (pair-concurrent MMs + some LDWEIGHTS overlap). PV per key tile:
    two [128,7]x[128,512] in col group 0 (serialized; full-row duty
    keeps HAM warm), PSUM start/stop over the 16 key tiles reduces for
    free. PVs trail their buffer by one iteration so they never
    head-block the strict-FIFO PE queue waiting on that buffer's exp.

  - Normalization is DECOUPLED from the main loop to kill cross-FIFO
    convoys: each chunk's raw ctx rows [7, 1024] go to an SBUF staging
    slot (ScalarE copy, which opens the next chunk on ACT); a "bundle"
    (8 tiny transpose+Wv matmuls into an already-consumed s slot, 2
    strided reciprocals, 2 broadcast multiplies, 1 contiguous bf16
    output DMA; host reorders during unshard) runs one chunk later,
    when all its deps are complete. Only the last copy+bundle drain
    after the loop.
"""

import math

import numpy as np
import ml_dtypes

import concourse.bass as bass
import concourse.bacc as bacc
import concourse.tile as tile
from concourse import mybir
from concourse.bass_utils import run_bass_kernel_spmd

f32 = mybir.dt.float32
bf16 = mybir.dt.bfloat16
i16 = mybir.dt.int16
EXP = mybir.ActivationFunctionType.Exp

B, H, S, D = 4, 8, 2048, 3
NCORES = 8
HPC = H // 2           # heads per core = 4
QCH = 512              # query chunk
NQ = S // QCH          # 4
KT = 128               # key tile
NKT = S // KT          # 16
NC4 = QCH // KT        # 128-query blocks per chunk = 4
NU = 2 * NKT           # units per (pair, chunk) = 32
NBUF = NU // 2         # s buffers per chunk = 16 (2 units each)
NCH = 2 * NQ           # chunks = 8
SCALE = 1.0 / math.sqrt(D)

# DVE-exp constants: bits16 = round(score*EXPC1 + EXPC2); bitcast bf16.
EXPC1 = SCALE * 128.0 / math.log(2.0)
EXPC2 = 127.0 * 128.0 - 5.568
# even buffers to DVE, odd to ACT: consecutive buffers overlap engines and
# ACT opens each chunk with the ctx->staging copy instead of an exp.
DVE_BUFS = frozenset(range(0, NBUF, 2))

# 3-way bf16 split product terms kept for q.k (drop (2,3),(3,2),(3,3))
Q_ORDER = (0, 0, 1, 0, 2, 1)
K_ORDER = (0, 1, 0, 2, 0, 1)


def _split3_bf16(a: np.ndarray):
    """3-way bf16 split: a ~= a1 + a2 + a3, each bf16."""
    a = np.ascontiguousarray(a, dtype=np.float32)
    a1 = a.astype(ml_dtypes.bfloat16)
    r = a - a1.astype(np.float32)
    a2 = r.astype(ml_dtypes.bfloat16)
    a3 = (r - a2.astype(np.float32)).astype(ml_dtypes.bfloat16)
    return a1, a2, a3


def _build_nc():
    nc = bacc.Bacc("TRN2", target_bir_lowering=False, debug=False,
                   num_devices=NCORES)

    qstk_in = [nc.dram_tensor(f"qstk{p}", [128, S], bf16,
                              kind="ExternalInput").ap() for p in range(2)]
    kstk_in = [nc.dram_tensor(f"kstk{p}", [128, S], bf16,
                              kind="ExternalInput").ap() for p in range(2)]
    xo_in = nc.dram_tensor("xo", [128, NKT, 7], bf16, kind="ExternalInput").ap()
    wv7_in = nc.dram_tensor("wv7", [128, 16], bf16, kind="ExternalInput").ap()
    # device-friendly layout [chunk, partition, c4, hl, d]: each bundle's
    # output DMA is fully contiguous; the host reorders during unshard
    out = nc.dram_tensor("out", [NCH, 128, NC4, 2, D], bf16,
                         kind="ExternalOutput").ap()

    with tile.TileContext(nc) as tc:
        with tc.tile_pool(name="per", bufs=1) as per, \
             tc.tile_pool(name="work", bufs=1) as work, \
             tc.tile_pool(name="spool", bufs=3, space="PSUM") as spool, \
             tc.tile_pool(name="cpool", bufs=1, space="PSUM") as cpool:
            qstk = [per.tile([128, S], bf16, name=f"qs{p}") for p in range(2)]
            kstk = [per.tile([128, S], bf16, name=f"ks{p}") for p in range(2)]
            xo = per.tile([128, NKT, 7], bf16)
            wv7 = per.tile([128, 16], bf16)
            # raw ctx rows for every chunk live here until its bundle runs
            staging = per.tile([128, NCH, 2 * QCH], bf16)

            # ACT exp-table preload overlaps the input DMAs
            tdum = per.tile([128, 1], f32)
            tdum2 = per.tile([128, 1], f32)
            nc.gpsimd.memset(tdum, 0.0)
            nc.scalar.activation(tdum2, tdum, EXP)

            # HAM warmup: ~4us of full-row dummy matmuls on a memset tile
            # flips the PE clock gate to 8/8 before the real work arrives
            # (a cold start costs ~13us: first ~26us ran at 1.2 GHz).
            warm = per.tile([128, 5 * KT], bf16)
            dummy = cpool.tile([128, 2 * QCH], f32, name="dummyctx",
                               tag="ctx")
            nc.gpsimd.memset(warm, 0.0)
            for i in range(6):
                nc.tensor.matmul(
                    dummy[:, 0:QCH], lhsT=warm[:, 0:KT], rhs=warm[:, KT:],
                    start=True, stop=True, tile_position=(0, 0))

            # pair-0 rows first so the first QKs can start early
            nc.sync.dma_start(out=kstk[0][0:64, :], in_=kstk_in[0][0:64, :])
            nc.sync.dma_start(out=qstk[0][0:64, 0:QCH],
                              in_=qstk_in[0][0:64, 0:QCH])
            nc.gpsimd.dma_start(out=kstk[0][64:128, :],
                                in_=kstk_in[0][64:128, :])
            nc.gpsimd.dma_start(out=wv7, in_=wv7_in)
            nc.sync.dma_start(out=qstk[0][64:128, 0:QCH],
                              in_=qstk_in[0][64:128, 0:QCH])
            nc.gpsimd.dma_start(out=xo, in_=xo_in)
            nc.sync.dma_start(out=qstk[0][0:64, QCH:],
                              in_=qstk_in[0][0:64, QCH:])
            nc.gpsimd.dma_start(out=qstk[0][64:128, QCH:],
                                in_=qstk_in[0][64:128, QCH:])
            nc.sync.dma_start(out=kstk[1], in_=kstk_in[1])
            nc.gpsimd.dma_start(out=qstk[1], in_=qstk_in[1])

            chunks = [(p, qc) for p in range(2) for qc in range(NQ)]
            pending = []  # deferred pieces; each takes the current s buffer

            def emit_buffer(p, qc, b):
                """QK matmuls for s-buffer b (2 units = 1 key tile) of chunk
                (p, qc). Row-group layout: even key tiles use groups {0,1},
                odd {2,3}, so the next buffer's LDWEIGHTS prefetches into
                idle groups while this buffer's matmuls stream."""
                s = spool.tile([128, 2 * QCH], f32, name=f"s{p}{qc}_{b}",
                               tag="s")
                t = b
                for hl in range(2):
                    base = 64 * (t % 2) + 32 * hl
                    nc.tensor.matmul(
                        s[:, hl * QCH:(hl + 1) * QCH],
                        lhsT=kstk[p][base:base + 32, t * KT:(t + 1) * KT],
                        rhs=qstk[p][base:base + 32,
                                    qc * QCH:(qc + 1) * QCH],
                        start=True, stop=True,
                        tile_position=(base, 0),
                    )
                return s

            def mk_copy(ci, _ctx):
                # on ACT: it opens each chunk (DVE owns buffer 0's exp), so
                # the next chunk's first PV never waits on this copy
                def go(s_exp):
                    nc.scalar.copy(
                        staging[0:7, ci, 0:2 * QCH], _ctx[0:7, 0:2 * QCH])
                return go

            def mk_bundle(src_ci):
                """Normalize + emit output for chunk src_ci, scratching its
                transpose blocks into an already-exp-consumed s slot."""
                p, qc = chunks[src_ci]

                def go(scr):
                    # ct blocks at cols QCH + 16*(2*c4+hl): [3 outs | denom]
                    ost = work.tile([128, NC4, 2, D], bf16,
                                    name=f"ost{src_ci}", tag="ost", bufs=2)
                    for c4 in range(NC4):
                        for hl in range(2):
                            base = QCH + 16 * (2 * c4 + hl)
                            nc.tensor.matmul(
                                scr[:, base:base + 4],
                                lhsT=staging[0:7, src_ci,
                                             hl * QCH + c4 * KT:
                                             hl * QCH + (c4 + 1) * KT],
                                rhs=wv7[0:7, 8 * p + 4 * hl:8 * p + 4 * hl + 4],
                                start=True, stop=True,
                                tile_position=(0, 0),
                            )
                    rec = work.tile([128, 2, NC4], f32,
                                    name=f"rec{src_ci}", tag="rec", bufs=2)
                    for hl in range(2):
                        base = QCH + 16 * hl
                        nc.vector.reciprocal(
                            rec[:, hl, :],
                            scr[:, base + 3:base + 100:32])
                    for hl in range(2):
                        base = QCH + 16 * hl
                        blk = scr[:, base:base + 128].rearrange(
                            "p (c s) -> p c s", s=32)
                        num = blk[:, :, 0:3]
                        sc = rec[:, hl, :].to_broadcast((128, NC4, D))
                        nc.vector.tensor_tensor(
                            out=ost[:, :, hl, :], in0=num, in1=sc,
                            op=mybir.AluOpType.mult)
                    dst = bass.AP(
                        tensor=out.tensor,
                        offset=src_ci * 128 * NC4 * 2 * D,
                        ap=[[NC4 * 2 * D, 128], [2 * D, NC4], [D, 2],
                            [1, D]],
                    )
                    nc.sync.dma_start(out=dst, in_=ost[:, :, :, :])
                return go

            def emit_pv(ctx, ptiles, t):
                for hl in range(2):
                    u = 2 * t + hl
                    ent = ptiles[u // 2]
                    psrc, is_i16 = ent[0], ent[1]
                    if len(ent) == 3 and hl == 1:
                        psrc, is_i16 = ent[2], False  # ACT half of buffer 0
                    rhs = psrc[:, (u % 2) * QCH:(u % 2 + 1) * QCH]
                    if is_i16:
                        rhs = rhs.bitcast(bf16)
                    nc.tensor.matmul(
                        ctx[0:7, hl * QCH:(hl + 1) * QCH],
                        lhsT=xo[:, t, :],
                        rhs=rhs,
                        start=(t == 0), stop=(t == NKT - 1),
                        tile_position=(0, 0),
                    )

            s_cur = emit_buffer(0, 0, 0)
            s_exp = None
            for ci, (p, qc) in enumerate(chunks):
                ctx = cpool.tile([128, 2 * QCH], f32, name=f"ctx{p}{qc}",
                                 tag="ctx")
                ptiles = {}
                for b in range(NBUF):
                    fsz = 2 * QCH
                    if b == 0 or b == NBUF - 1:
                        # boundary-critical buffers: halve the exp latency
                        # by splitting across both engines (different PSUM
                        # banks -> truly parallel). Around each chunk
                        # boundary the engines each carry ~1.34us instead of
                        # 1.77us serial exp, so PV(15) and PV(b0) start early
                        pt = work.tile([128, 2 * QCH], i16,
                                       name=f"pi{p}{qc}_{b}", tag="pi",
                                       bufs=5)
                        pt2 = work.tile([128, 2 * QCH], bf16,
                                        name=f"p{p}{qc}_{b}", tag="p",
                                        bufs=5)
                        ptiles[b] = (pt, True, pt2)
                        nc.vector.tensor_scalar(
                            out=pt[:, 0:QCH], in0=s_cur[:, 0:QCH],
                            scalar1=EXPC1, scalar2=EXPC2,
                            op0=mybir.AluOpType.mult,
                            op1=mybir.AluOpType.add)
                        nc.scalar.activation(pt2[:, QCH:fsz],
                                             s_cur[:, QCH:fsz],
                                             EXP, scale=SCALE)
                    elif b in DVE_BUFS:
                        pt = work.tile([128, 2 * QCH], i16,
                                       name=f"pi{p}{qc}_{b}", tag="pi",
                                       bufs=5)
                        ptiles[b] = (pt, True)
                        nc.vector.tensor_scalar(
                            out=pt[:, 0:fsz], in0=s_cur[:, 0:fsz],
                            scalar1=EXPC1, scalar2=EXPC2,
                            op0=mybir.AluOpType.mult,
                            op1=mybir.AluOpType.add)
                    else:
                        pt = work.tile([128, 2 * QCH], bf16,
                                       name=f"p{p}{qc}_{b}", tag="p", bufs=5)
                        ptiles[b] = (pt, False)
                        nc.scalar.activation(pt[:, 0:fsz], s_cur[:, 0:fsz],
                                             EXP, scale=SCALE)
                    s_prev = s_exp  # slot of buffer b-1: exp long done
                    s_exp = s_cur
                    if b + 1 < NBUF:
                        s_cur = emit_buffer(p, qc, b + 1)
                    elif ci + 1 < len(chunks):
                        s_cur = emit_buffer(*chunks[ci + 1], 0)
                    else:
                        s_cur = None
                    # pieces fire on even (DVE-exp) buffers: the copy opens
                    # ACT's chunk; the bundle's deps are a chunk old by then
                    if pending and b % 2 == 0 and s_prev is not None:
                        pending.pop(0)(s_prev)
                    # PV trails by one buffer so it never head-blocks the
                    # PE queue waiting on this buffer's exp
                    if b > 0:
                        emit_pv(ctx, ptiles, b - 1)

                emit_pv(ctx, ptiles, NBUF - 1)
                pending.append(mk_copy(ci, ctx))
                pending.append(mk_bundle(ci))

            # drain: the last chunk's copy + bundle on one fresh s slot
            sx = spool.tile([128, 2 * QCH], f32, name="sx", tag="s")
            while pending:
                pending.pop(0)(sx)

    nc.compile()
    return nc


_NC_CACHE = None


def _get_nc():
    global _NC_CACHE
    if _NC_CACHE is None:
        _NC_CACHE = _build_nc()
    return _NC_CACHE


def _make_in_maps(x, W_query, W_key, W_value):
    in_maps = []
    for c in range(NCORES):
        b = c // 2
        hp = (c % 2) * HPC
        xb = x[b, 0]                                    # [S, 3]

        # per-pair stacks; each head's 18 split rows are duplicated into
        # two 32-row groups (64*grp + 32*hl) so even/odd key tiles hit
        # disjoint PE row groups (LDWEIGHTS prefetch overlap)
        qstk = [np.zeros((128, S), dtype=ml_dtypes.bfloat16)
                for _ in range(2)]
        kstk = [np.zeros((128, S), dtype=ml_dtypes.bfloat16)
                for _ in range(2)]
        for h in range(HPC):
            pq, hl = h // 2, h % 2
            Qh = (xb @ W_query[0, hp + h]).T            # [3, S]
            Kh = (xb @ W_key[0, hp + h]).T
            qp = _split3_bf16(Qh)
            kp = _split3_bf16(Kh)
            for t6 in range(6):
                for grp in range(2):
                    r = 64 * grp + 32 * hl + 3 * t6
                    qstk[pq][r:r + 3] = qp[Q_ORDER[t6]]
                    kstk[pq][r:r + 3] = kp[K_ORDER[t6]]

        # xo[p, t, :] = [x_hi(3) | x_lo(3) | 1] at position t*128+p.
        xh = xb.astype(ml_dtypes.bfloat16)
        xl = (xb - xh.astype(np.float32)).astype(ml_dtypes.bfloat16)
        xo = np.concatenate(
            [xh, xl, np.ones((S, 1), ml_dtypes.bfloat16)], axis=1)
        xo = np.ascontiguousarray(
            xo.reshape(NKT, 128, 7).transpose(1, 0, 2))

        # wv7 block for head 2p+hl at partitions 0:7, columns 8p+4hl:
        # rows [Wv; Wv; denom-selector]
        wv7 = np.zeros((128, 16), ml_dtypes.bfloat16)
        for h in range(HPC):
            Wv = W_value[0, hp + h]                     # [3, 3]
            wc = 8 * (h // 2) + 4 * (h % 2)
            wv7[0:3, wc:wc + 3] = Wv
            wv7[3:6, wc:wc + 3] = Wv
            wv7[6, wc + 3] = 1.0

        in_maps.append({
            "qstk0": qstk[0],
            "qstk1": qstk[1],
            "kstk0": kstk[0],
            "kstk1": kstk[1],
            "xo": xo,
            "wv7": wv7,
        })
    return in_maps


def kernel(x, W_query, W_key, W_value, _trace=False, _tmpdir=None):
    x = np.asarray(x, dtype=np.float32)
    W_query = np.asarray(W_query, dtype=np.float32)
    W_key = np.asarray(W_key, dtype=np.float32)
    W_value = np.asarray(W_value, dtype=np.float32)

    nc = _get_nc()
    res = run_bass_kernel_spmd(
        nc,
        _make_in_maps(x, W_query, W_key, W_value),
        core_ids=list(range(NCORES)),
        trace=_trace,
        tmpdir=_tmpdir,
    )
    full = np.empty((B, H, S, D), dtype=np.float32)
    for c in range(NCORES):
        b = c // 2
        hp = (c % 2) * HPC
        o = np.asarray(res.results[c]["out"]).astype(np.float32)
        # [ci, part, c4, hl, d] -> [head, q, d]
        o = o.reshape(2, NQ, 128, NC4, 2, D)
        full[b, hp:hp + HPC] = o.transpose(0, 4, 1, 3, 2, 5).reshape(
            HPC, S, D)
    if _trace:
        kernel._last_results = res
    return (full, full)
